# revision 16
# baseline (speedup 1.0000x reference)
"""Trainium2 Bass kernel for nn_ACmix_49658411876423.

Math notes (verified vs reference, rel err ~3.6e-3):

The reference's attention mask is inverted (valid key positions are set to
-FLT_MAX, zero-padded halo positions keep their logits).  Consequences:
  * Interior windows: uniform softmax -> output = mean of v over the 12x12
    window.  Boundary windows: all weight on zero-padded v -> exactly 0.
So the attention branch is:  out_att = (rate1/144) * W3 @ (12x12 window
sums of x) + rate1*b3 on interior blocks, 0 on boundary blocks.

The conv branch is affine in x with weights folded host-side into 9 dense
A[ky,kx] (256x256) matrices plus a rank-10 bias map.

Device kernel per core (8 batches, data-parallel over 8 cores), v3:
  * 1-D Winograd F(2,3) along x: the 3 kx taps become 4 transform points,
    each output-column PAIR shares the pt1/pt2/pt3 products.  Conv matmul
    columns drop 3x -> 2x vs the direct algorithm (the PE stream is the
    wall, 99% dense in the direct version).
      V[pt] (forward transform, 4 DVE adds per batch):
        V0=d0-d2, V1=d1+d2, V2=d2-d1, V3=d1-d3   (d_j = xpad[:, 2tx+j])
      m[pt] = sum_ky U[ky,pt] @ V[pt] (PSUM, one bank per pt)
      out_even = m0+m1+m2 ; out_odd = m1-m2-m3   (inverse transform)
  * x is staged host-side in an even/odd/shifted column layout
    [E16|E'16|O16|O'16] per row so every V op reads stride-1 4B-aligned
    bf16 runs -> DVE 2x packed mode.
  * V1 doubles as the first stage of the attention window sums
    (V1[tx] = x[2tx]+x[2tx+1]), collapsing 3 DVE ops into 1.
  * The attention values and the rank-10 bias map are injected in the
    TRANSFORM domain: a K=35 mask matmul adds (att+bias at even x) into the
    m0 bank and -(att+bias at odd x) into the m3 bank; the inverse
    transform then reproduces them exactly in the output.
  * Inverse transform drain spread over three engines: ScalarE copies m1,
    DVE copies m2 and does the PSUM-operand adds, GpSimd does the SBUF-only
    subtract.  V/wsum for batch b+1 are emitted before b's drains so the
    PE never waits on the forward transform.
  * ky=0 / ky=2 taps skip the output row that only reads zero padding
    (N=378 instead of 392).
"""

import numpy as np

import concourse.bass as bass
import concourse.mybir as mybir
import concourse.tile as tile
from concourse import bacc
from concourse.bass_utils import run_bass_kernel_spmd

B, C, H, W = 64, 256, 28, 28
HEAD_DIM = 64
BLOCK, HALO, WIN, KCONV = 4, 4, 12, 3
NB = H // BLOCK          # 7 blocks per side
NI = NB - 2              # 5 interior blocks per side
NCORES = 8
BLOC = B // NCORES       # 8 batches per core
HP = H + 2               # 30 padded rows
XW = 64                  # row stride in the E/E'/O/O' staged layout
HW = H * W               # 784
TX = W // 2              # 14 winograd column pairs
NW = H * TX              # 392 columns per (pt) matmul (both row halves)
KM = 25 + 10             # mask-matmul contraction: 25 att rows + 10 bias rows

F32 = mybir.dt.float32
BF16 = mybir.dt.bfloat16

# set by test harness to enable NTFF profiling
TRACE = False
last_exec_time_ns = None

_graph_cache = None


def _build_graph():
    nc = bacc.Bacc("TRN2", target_bir_lowering=False, debug=False,
                   num_devices=NCORES)

    xp_d = nc.dram_tensor("xp", [BLOC, C, HP * XW], BF16,
                          kind="ExternalInput").ap()
    wg_d = nc.dram_tensor("wg", [128, 3, 4, 2, C], BF16,
                          kind="ExternalInput").ap()
    w3_d = nc.dram_tensor("w3p", [128, 2, C], BF16, kind="ExternalInput").ap()
    mk_d = nc.dram_tensor("mk", [KM, 2 * NW], BF16, kind="ExternalInput").ap()
    bl_d = nc.dram_tensor("bl", [10, C], BF16, kind="ExternalInput").ap()
    out_d = nc.dram_tensor("out", [BLOC, C, HW], F32, kind="ExternalOutput").ap()

    with tile.TileContext(nc) as tc:
        with (
            tc.tile_pool(name="wconst", bufs=1) as wconst,
            tc.tile_pool(name="xpool", bufs=4) as xpool,
            tc.tile_pool(name="vpool", bufs=3) as vpool,
            tc.tile_pool(name="spool", bufs=4) as spool,
            tc.tile_pool(name="rpool", bufs=2) as rpool,
            tc.tile_pool(name="apool", bufs=8) as apool,
            tc.tile_pool(name="opool", bufs=3) as opool,
            tc.tile_pool(name="ppool", bufs=7, space="PSUM") as ppool,
            tc.tile_pool(name="patt", bufs=1, space="PSUM") as patt,
        ):
            def load_xt(b):
                xt = xpool.tile([128, 2, HP * XW], BF16, tag="xp",
                                name=f"xt{b % 4}")
                for kt in range(2):
                    nc.sync.dma_start(
                        out=xt[:, kt, :],
                        in_=xp_d[b, kt * 128:(kt + 1) * 128, :])
                return xt

            wg_sb = wconst.tile([128, 3, 4, 2, C], BF16)
            w3_sb = wconst.tile([128, 2, C], BF16)
            mk_sb = wconst.tile([KM, 2 * NW], BF16)
            attL = [wconst.tile([KM, C], BF16, name=f"attL{i}")
                    for i in range(2)]

            # critical path first: x(b0), then ALL weights (they are needed
            # within ~10us and must not queue behind x(b1)'s 1MB), then x(b1)
            xts = {}
            # b0: two separate per-kt tiles so the kt0 forward transform
            # (and with it the first matmul) starts after only 480KB of DMA
            xt0a = wconst.tile([128, HP * XW], BF16, name="xt0a")
            xt0b = wconst.tile([128, HP * XW], BF16, name="xt0b")
            nc.sync.dma_start(out=xt0a[:], in_=xp_d[0, 0:128, :])
            nc.sync.dma_start(out=wg_sb[:, 0, 0, :, :], in_=wg_d[:, 0, 0, :, :])
            nc.sync.dma_start(out=xt0b[:], in_=xp_d[0, 128:256, :])
            for pt in range(1, 4):
                nc.sync.dma_start(out=wg_sb[:, 0, pt, :, :],
                                  in_=wg_d[:, 0, pt, :, :])
            for ky in range(1, 3):
                nc.sync.dma_start(out=wg_sb[:, ky, :, :, :],
                                  in_=wg_d[:, ky, :, :, :])
            nc.sync.dma_start(out=w3_sb[:], in_=w3_d[:])
            xts[1] = load_xt(1)
            nc.sync.dma_start(out=mk_sb[:], in_=mk_d[:])
            for i in range(2):
                nc.sync.dma_start(out=attL[i][25:, :], in_=bl_d[:])

            def fwd_transform(b, xt, xt1=None):
                """V[kt, pt, row(30), tx(14)] from the E/E'/O/O' layout.
                All reads are stride-1 14-element 16B-aligned runs.
                xt1: separate kt1 tile (b0 startup only) - per-kt ops, kt0
                first, so the first matmuls start before x kt1 lands."""
                vt = vpool.tile([128, 2, 4, 30 * TX], BF16, tag="v",
                                name=f"v{b % 3}")
                if xt1 is None:
                    groups = (((0, 1), xt[:], 0),)
                else:
                    groups = (((0,), xt[:], 0), ((1,), xt1[:], 0))

                for ktg, base, boff in groups:
                    k0 = ktg[0]

                    def eo(block):
                        return bass.AP(
                            tensor=base.tensor,
                            offset=base.offset + boff + block * 16,
                            ap=[list(base.ap[0]), [HP * XW, len(ktg)],
                                [XW, HP], [1, TX]])

                    def vout(pt):
                        vb = vt[:]
                        return bass.AP(
                            tensor=vb.tensor,
                            offset=vb.offset + k0 * 4 * 30 * TX
                            + pt * 30 * TX,
                            ap=[list(vb.ap[0]), [4 * 30 * TX, len(ktg)],
                                [TX, HP], [1, TX]])

                    E, Es, O, Os = eo(0), eo(1), eo(2), eo(3)
                    nc.vector.tensor_sub(out=vout(0), in0=E, in1=Es)
                    nc.vector.tensor_add(out=vout(1), in0=O, in1=Es)
                    nc.vector.tensor_sub(out=vout(2), in0=Es, in1=O)
                    nc.vector.tensor_sub(out=vout(3), in0=O, in1=Os)
                return vt

            def window_sums(b, vt):
                """12x12 window sums at the 5x5 interior blocks from V1
                (V1[tx] = x[2tx] + x[2tx+1]) -> r3 [128, 2, 25] bf16."""
                vb = vt[:]

                def v1tap(par):
                    # V1 plane rows 1..28 (x rows 0..27), col pairs
                    return bass.AP(
                        tensor=vb.tensor,
                        offset=vb.offset + 1 * 30 * TX + TX + par,
                        ap=[list(vb.ap[0]), [4 * 30 * TX, 2], [TX, H],
                            [2, NB]])

                s1 = spool.tile([128, 2, H * NB], F32, tag="s1")
                nc.vector.tensor_add(out=s1[:], in0=v1tap(0), in1=v1tap(1))

                def s1tap(dy):
                    sb = s1[:]
                    return bass.AP(
                        tensor=sb.tensor, offset=sb.offset + dy * NB,
                        ap=[list(sb.ap[0]), [H * NB, 2], [BLOCK * NB, NB],
                            [1, NB]])

                t2a = spool.tile([128, 2, NB * NB], F32, tag="t2a")
                t2b = spool.tile([128, 2, NB * NB], F32, tag="t2b")
                s2 = spool.tile([128, 2, NB * NB], F32, tag="s2")
                nc.vector.tensor_add(out=t2a[:], in0=s1tap(0), in1=s1tap(1))
                nc.vector.tensor_add(out=t2b[:], in0=s1tap(2), in1=s1tap(3))
                nc.vector.tensor_add(out=s2[:], in0=t2a[:], in1=t2b[:])

                def s2tap(j):
                    sb = s2[:]
                    return bass.AP(
                        tensor=sb.tensor, offset=sb.offset + j,
                        ap=[list(sb.ap[0]), [NB * NB, 2], [NB, NB], [1, NI]])

                u1 = spool.tile([128, 2, NB * NI], F32, tag="u1")
                u2 = spool.tile([128, 2, NB * NI], F32, tag="u2")
                nc.vector.tensor_add(out=u1[:], in0=s2tap(0), in1=s2tap(1))
                nc.vector.tensor_add(out=u2[:], in0=u1[:], in1=s2tap(2))

                def utap(i):
                    ub = u2[:]
                    return bass.AP(
                        tensor=ub.tensor, offset=ub.offset + i * NI,
                        ap=[list(ub.ap[0]), [NB * NI, 2], [NI, NI], [1, NI]])

                v1 = spool.tile([128, 2, 25], F32, tag="v1")
                r3 = rpool.tile([128, 2, 25], BF16, tag="r3",
                                name=f"r3{b % 2}")
                nc.vector.tensor_add(out=v1[:], in0=utap(0), in1=utap(1))
                nc.vector.tensor_add(out=r3[:], in0=v1[:], in1=utap(2))
                return r3

            # preamble: V + window sums for b0 (DVE), while weights stream in
            vts = {0: fwd_transform(0, xt0a, xt0b)}
            r3s = {0: window_sums(0, vts[0])}

            mkb = mk_sb[:]
            for b in range(BLOC):
                if b + 2 < BLOC:
                    xts[b + 2] = load_xt(b + 2)
                vt = vts.pop(b)
                vb = vt[:]

                def vrhs(kt, pt, row0, nrow):
                    return bass.AP(
                        tensor=vb.tensor,
                        offset=vb.offset + kt * 4 * 30 * TX + pt * 30 * TX
                        + row0 * TX,
                        ap=[list(vb.ap[0]), [TX, nrow], [1, TX]])

                for mt in range(2):
                    ms = slice(mt * 128, (mt + 1) * 128)
                    pts = [ppool.tile([128, NW], F32, tag="pc",
                                      name=f"pc{pt}", padded_shape=[128, 512])
                           for pt in range(4)]
                    # ky=0: out rows 1..27 (row 0 only reads zero pad);
                    # ky=1: rows 0..27; ky=2: rows 0..26
                    # ky=0: out rows 1..27 (row 0 only reads zero pad);
                    # ky=1: rows 0..27; ky=2: rows 0..26.
                    # First tile set: kt-major / ky-inner so each V plane
                    # (produced every ~500ns by the DVE at startup) feeds 3
                    # matmuls while x kt1 is still in flight.
                    KYP = {0: (1, 27, 14), 1: (1, 28, 0), 2: (2, 27, 0)}
                    if b == 0 and mt == 0:
                        order = [(ky, pt, kt) for kt in range(2)
                                 for pt in range(4) for ky in range(3)]
                    else:
                        order = [(ky, pt, kt) for ky in range(3)
                                 for pt in range(4) for kt in range(2)]
                    for ky, pt, kt in order:
                        row0, nrow, oc0 = KYP[ky]
                        pb = pts[pt][:]
                        outap = bass.AP(
                            tensor=pb.tensor, offset=pb.offset + oc0,
                            ap=[list(pb.ap[0]), [1, nrow * TX]])
                        nc.tensor.matmul(
                            outap, wg_sb[:, ky, pt, kt, ms],
                            vrhs(kt, pt, row0, nrow),
                            start=(ky == 0 and kt == 0),
                            stop=(ky == 2 and kt == 1 and pt in (1, 2)))

                    if mt == 0:
                        # att projection for this batch (window sums were
                        # emitted one iteration earlier)
                        paT = patt.tile([25, C], F32, tag="paT",
                                        padded_shape=[25, 512])
                        r3b = r3s[b][:]
                        for kt in range(2):
                            r3k = bass.AP(tensor=r3b.tensor,
                                          offset=r3b.offset + kt * 25,
                                          ap=[list(r3b.ap[0]), [1, 25]])
                            nc.tensor.matmul(paT[:], r3k, w3_sb[:, kt, :],
                                             start=(kt == 0), stop=(kt == 1))
                        nc.scalar.copy(out=attL[b % 2][0:25, :], in_=paT[:])
                        del r3s[b]
                        # forward transform + window sums for b+1 (DVE runs
                        # these while the PE streams this batch's matmuls)
                        if b + 1 < BLOC:
                            vts[b + 1] = fwd_transform(b + 1, xts[b + 1])
                            r3s[b + 1] = window_sums(b + 1, vts[b + 1])

                    # att + bias injected in transform domain: even cols
                    # into m0, -(odd cols) into m3
                    for which, pt in ((0, 0), (1, 3)):
                        nc.tensor.matmul(
                            pts[pt][:], attL[b % 2][:, ms],
                            bass.AP(tensor=mkb.tensor,
                                    offset=mkb.offset + which * NW,
                                    ap=[list(mkb.ap[0]), [1, NW]]),
                            start=False, stop=True)

                    # ---- inverse transform: out_e = m0+m1+m2,
                    #      out_o = m1-m2-m3, spread across Scalar/DVE/GpSimd
                    c1 = apool.tile([128, NW], F32, tag="c1")
                    c2 = apool.tile([128, NW], F32, tag="c2")
                    te = apool.tile([128, NW], F32, tag="te")
                    uo = apool.tile([128, NW], F32, tag="uo")
                    nc.scalar.copy(out=c1[:], in_=pts[1][:])
                    nc.scalar.copy(out=c2[:], in_=pts[2][:])
                    nc.gpsimd.tensor_sub(out=uo[:], in0=c1[:], in1=c2[:])
                    nc.vector.tensor_add(out=te[:], in0=pts[0][:], in1=c1[:])

                    out_sb = opool.tile([128, HW], F32, tag="osb",
                                        name=f"osb{b % 3}")
                    ob = out_sb[:]

                    def oap(e):
                        return bass.AP(
                            tensor=ob.tensor, offset=ob.offset + e,
                            ap=[list(ob.ap[0]), [W, H], [2, TX]])

                    nc.vector.tensor_add(out=oap(0), in0=te[:], in1=c2[:])
                    nc.vector.tensor_sub(out=oap(1), in0=uo[:], in1=pts[3][:])
                    nc.sync.dma_start(out=out_d[b, ms, :], in_=out_sb[:])

    nc.compile()
    return nc


def _host_precompute(w1, b1, w2, b2, w3, b3, fc_w, dep_w, rate1, rate2):
    """Fold all the small parameters into the Winograd-transformed conv
    matrices, the window-attention projection, and the bias-map factors."""
    f64 = np.float64
    Wsrc = [w1.astype(f64), w2.astype(f64), w3.astype(f64)]
    bsrc = [b1.astype(f64), b2.astype(f64), b3.astype(f64)]
    fc = fc_w.astype(f64)
    dw = dep_w.astype(f64)
    r1 = float(rate1[0])
    r2 = float(rate2[0])

    M9 = np.zeros((9, HEAD_DIM, C), f64)
    B9 = np.zeros((9, HEAD_DIM), f64)
    for i in range(9):
        for c in range(12):
            h = c % 4
            M9[i] += fc[i, c] * Wsrc[c // 4][h * 64:(h + 1) * 64, :]
            B9[i] += fc[i, c] * bsrc[c // 4][h * 64:(h + 1) * 64]

    g = np.arange(C) // 4
    A = np.zeros((9, C, C), f64)      # A[s = ky*3+kx]
    bA = np.zeros((9, C), f64)
    for ky in range(3):
        for kx in range(3):
            s = ky * 3 + kx
            A[s] = r2 * np.einsum('oi,ioc->oc', dw[:, :, ky, kx], M9[:, g, :])
            bA[s] = r2 * np.einsum('oi,io->o', dw[:, :, ky, kx], B9[:, g])

    # Winograd F(2,3) weights along x: U[ky,0]=A0, U[ky,1]=(A0+A1+A2)/2,
    # U[ky,2]=(A0-A1+A2)/2, U[ky,3]=A2; lhsT layout [kp, ky, pt, kt, oc]
    wg = np.empty((128, 3, 4, 2, C), np.float32)
    for ky in range(3):
        A0, A1, A2 = A[ky * 3 + 0], A[ky * 3 + 1], A[ky * 3 + 2]
        U = (A0, (A0 + A1 + A2) / 2, (A0 - A1 + A2) / 2, A2)
        for pt in range(4):
            Ut = U[pt].T.astype(np.float32)            # [cin, oc]
            for kt in range(2):
                wg[:, ky, pt, kt, :] = Ut[kt * 128:(kt + 1) * 128, :]

    # attention projection, (rate1/144)-folded, [k-part, k-tile, oc]
    w3p = np.empty((128, 2, C), np.float32)
    w3t = ((r1 / 144.0) * Wsrc[2]).T.astype(np.float32)
    for kt in range(2):
        w3p[:, kt, :] = w3t[kt * 128:(kt + 1) * 128, :]

    # mask rhs, sampled at even / negated-odd x columns: [35, 2, 28*14]
    yy, xx = np.meshgrid(np.arange(H), np.arange(W), indexing='ij')
    by, bx = yy // BLOCK, xx // BLOCK
    mk = np.zeros((KM, H, W), np.float32)
    for wy in range(NI):
        for wx in range(NI):
            mk[wy * NI + wx] = ((by == wy + 1) & (bx == wx + 1))
    for ky in range(3):
        for kx in range(3):
            ok_y = np.ones(H, bool)
            if ky == 0:
                ok_y[0] = False
            if ky == 2:
                ok_y[H - 1] = False
            ok_x = np.ones(W, bool)
            if kx == 0:
                ok_x[0] = False
            if kx == 2:
                ok_x[W - 1] = False
            mk[25 + ky * 3 + kx] = ok_y[:, None] & ok_x[None, :]
    interior = (by >= 1) & (by <= NB - 2) & (bx >= 1) & (bx <= NB - 2)
    mk[34] = interior
    mk2 = np.empty((KM, 2, H * TX), np.float32)
    mk2[:, 0, :] = mk[:, :, 0::2].reshape(KM, H * TX)
    mk2[:, 1, :] = -mk[:, :, 1::2].reshape(KM, H * TX)
    mk2 = mk2.reshape(KM, 2 * H * TX)

    # bias lhsT rows [10, 256]: rate2-folded tap biases + att bias
    bl = np.empty((10, C), np.float32)
    for s in range(9):
        bl[s] = bA[s].astype(np.float32)
    bl[9] = (r1 * bsrc[2]).astype(np.float32)

    return wg, w3p, mk2, bl


def _stage_x(x):
    """Host-side LAYOUT-ONLY staging of x (pad + even/odd split + shifted
    duplicates; no arithmetic): rows [E16 | E'=E<<1 | O16 | O'=O<<1]."""
    xpad = np.pad(np.ascontiguousarray(x, dtype=np.float32),
                  ((0, 0), (0, 0), (1, 1), (1, 3)))     # [B, C, 30, 32]
    st = np.zeros((B, C, HP, XW), np.float32)
    E = xpad[:, :, :, 0::2]                             # 16 even cols
    O = xpad[:, :, :, 1::2]                             # 16 odd cols
    st[:, :, :, 0:16] = E
    st[:, :, :, 16:31] = E[:, :, :, 1:]                 # E' (shift by one)
    st[:, :, :, 32:48] = O
    st[:, :, :, 48:63] = O[:, :, :, 1:]                 # O'
    return st.reshape(B, C, HP * XW)


def kernel(x, w1, b1, w2, b2, w3, b3, fc_w, dep_w, rel_height, rel_width,
           rate1, rate2):
    global _graph_cache, last_exec_time_ns
    if _graph_cache is None:
        _graph_cache = _build_graph()
    nc = _graph_cache

    wg, w3p, mk2, bl = _host_precompute(
        w1, b1, w2, b2, w3, b3, fc_w, dep_w, rate1, rate2)

    import ml_dtypes
    bf = ml_dtypes.bfloat16
    wg = wg.astype(bf)
    w3p = w3p.astype(bf)
    mk2 = mk2.astype(bf)
    bl = bl.astype(bf)
    xst = _stage_x(x).astype(bf)

    in_maps = []
    for i in range(NCORES):
        in_maps.append({
            "xp": np.ascontiguousarray(xst[i * BLOC:(i + 1) * BLOC]),
            "wg": wg, "w3p": w3p, "mk": mk2, "bl": bl,
        })

    kw = {}
    if TRACE:
        import tempfile
        kw["tmpdir"] = tempfile.mkdtemp(prefix="ktrace_", dir="/tmp")
        globals()["last_trace_dir"] = kw["tmpdir"]
    res = run_bass_kernel_spmd(nc, in_maps, core_ids=list(range(NCORES)),
                               trace=TRACE, **kw)
    last_exec_time_ns = res.exec_time_ns
    out = np.concatenate([res.results[i]["out"] for i in range(NCORES)], axis=0)
    return out.reshape(B, C, H, W)
